# revision 5
# baseline (speedup 1.0000x reference)
"""Differentiable Gaussian-splat tile compositor on 8 Trainium2 cores, v2.

Sharding: image split into 8 horizontal bands (32 rows), one per NeuronCore.
Within a band, 8x8 pixel tiles (128 tiles); each Gaussian is assigned
(host-side, exact per-pixel-center test) to the tiles where it can reach
alpha >= 1/255 (q <= 2*ln 255). Tiles are bin-packed into NS sweeps of <=128
depth-ordered gaussian rows and <= SLOTCAP tiles; all per-sweep layout is
padded to SLOTCAP so the device program is identical across cores (SPMD) --
the block-diagonal strict-lower mask realizing each tile's exclusive
cumulative sum of ln(1-alpha) is DMA'd as data.

Sweeps are batched into groups (<=512 PSUM cols). Per group g:
  q[g,pix] = A_s[12,128]^T @ B[12,64]   per sweep (PE, bf16 hi/lo split; the
                                         8x8 tile-local basis is exact bf16)
  m        = q <= 2 ln 255              (DVE, runs parallel with Exp)
  e        = exp(-q/2)                  (ACT)
  alpha    = min(e,.99) * m             (DVE fused)
  l        = ln(1 - alpha)              (ACT free affine, bf16 out)
  Tlog     = StrictLowerBlockDiag @ l   (PE per sweep, bf16)
  T        = exp(Tlog)                  (ACT)
  w        = alpha * T                  (Pool, bf16 out)
  img      = Colors_s^T @ w             (PE per sweep, partition-offset rows)
Per img chunk (<=128 output rows): evacuation copy + output DMA, overlapped
with later groups. Emission is software-pipelined (stage skew) so each
engine queue has lookahead; groups are sized small-big-small so the fill and
tail of the ACT-bound stream stay short.
"""

import os
import numpy as np

_H = 256
_W = 256
_NCORES = 8
_TH = 8
_TW = 8
_AREA = _TH * _TW
_BAND = _H // _NCORES
_TILES_X = _W // _TW
_TILES_Y = _BAND // _TH
_NTILES = _TILES_X * _TILES_Y
_CAP = 128
_QTH = float(2.0 * np.log(255.0))
_PAD_Q = 128.0


def _bf16dt():
    import ml_dtypes

    return ml_dtypes.bfloat16


def _reference_numpy(means_2d, covs_2d, depth_features, color_features, H, W):
    """Exact slow fallback (mirrors reference.py math)."""
    order = np.argsort(depth_features, kind="stable")
    m = means_2d[order].astype(np.float32)
    cv = covs_2d[order].astype(np.float32)
    cl = color_features[order].astype(np.float32)
    a, b, c = cv[:, 0], cv[:, 1], cv[:, 2]
    det = a * c - b * b
    ia, ib, ic = c / det, -b / det, a / det
    xs = np.arange(W, dtype=np.float32) + 0.5
    ys = np.arange(H, dtype=np.float32) + 0.5
    img = np.zeros((3, H, W), np.float32)
    T = np.ones((H, W), np.float32)
    for p in range(m.shape[0]):
        dx = xs[None, :] - m[p, 0]
        dy = ys[:, None] - m[p, 1]
        q = ia[p] * dx * dx + 2.0 * ib[p] * dx * dy + ic[p] * dy * dy
        alpha = np.minimum(np.float32(0.99), np.exp(np.float32(-0.5) * q))
        alpha = np.where(alpha < 1.0 / 255.0, np.float32(0.0), alpha)
        w = alpha * T
        img += cl[p][:, None, None] * w[None]
        T = T * (1.0 - alpha)
    return img


def _prep_core(core, m, ia, ib, ic, rx, ry):
    """Per-tile depth-ordered gaussian index lists for one core band."""
    tiles = []
    y_base = core * _BAND
    for ty in range(_TILES_Y):
        y0 = y_base + ty * _TH
        for tx in range(_TILES_X):
            x0 = tx * _TW
            cand = np.nonzero(
                (m[:, 0] + rx >= x0 + 0.5 - 1e-6)
                & (m[:, 0] - rx <= x0 + _TW - 0.5 + 1e-6)
                & (m[:, 1] + ry >= y0 + 0.5 - 1e-6)
                & (m[:, 1] - ry <= y0 + _TH - 0.5 + 1e-6)
            )[0]
            if cand.size:
                dx = (x0 + 0.5 + np.arange(_TW))[None, :] - m[cand, 0][:, None]
                dy = (y0 + 0.5 + np.arange(_TH))[None, :] - m[cand, 1][:, None]
                q = (
                    ia[cand][:, None, None] * (dx * dx)[:, None, :]
                    + 2.0 * ib[cand][:, None, None]
                    * dx[:, None, :] * dy[:, :, None]
                    + ic[cand][:, None, None] * (dy * dy)[:, :, None]
                )
                qmin = q.reshape(cand.size, -1).min(axis=1)
                cand = cand[qmin <= _QTH + 1e-3]
            tiles.append(cand)
    return tiles


def _pack_tiles(tiles, n_sweeps, slotcap):
    """FFD into exactly n_sweeps bins of <=_CAP rows, <=slotcap tiles.
    Returns sweeps: list of lists of (tile, idx, start_row)."""
    order = sorted(range(len(tiles)), key=lambda i: -len(tiles[i]))
    sweeps = [[] for _ in range(n_sweeps)]
    loads = [0] * n_sweeps
    for i in order:
        n = len(tiles[i])
        # best-fit: fullest bin that still fits (rows); balances slots via cap
        best, best_load = None, -1
        for s in range(n_sweeps):
            if loads[s] + n <= _CAP and len(sweeps[s]) < slotcap:
                if loads[s] > best_load:
                    best, best_load = s, loads[s]
        if best is None:
            raise ValueError("packing overflow")
        sweeps[best].append((i, tiles[i], loads[best]))
        loads[best] += n
    return sweeps


def _choose_shape(per_core_tiles):
    """Smallest (n_sweeps, slotcap) feasible for every core."""
    lo = max(
        max(1, -(-sum(len(t) for t in tiles) // _CAP))
        for tiles in per_core_tiles
    )
    for ns in range(lo, 65):
        for cap in range(-(-_NTILES // ns), 43):
            try:
                for tiles in per_core_tiles:
                    _pack_tiles(tiles, ns, cap)
                return ns, cap
            except ValueError:
                continue
    raise ValueError("unpackable")


def _split_groups(ns):
    """Consecutive groups of <=8 sweeps (<=512 PSUM cols). First and last
    are a single sweep: the first keeps the initial Exp's wait inline (so
    the act-table load stays at the head of the ACT queue) and starts the
    stream as soon as the table lands; the last keeps the tail chain
    short."""
    ov = os.environ.get("GS_GROUPS")
    if ov:
        groups = [int(x) for x in ov.split(",")]
        if sum(groups) == ns and all(1 <= g <= 8 for g in groups):
            return groups
    if ns <= 2:
        return [ns]
    if ns <= 5:
        return [1] + [ns - 2] + [1]
    # head group of 1 (fill), tail group of 2 (short drain chain),
    # middles of <=7 sweeps (balance per-op overhead vs pipeline slack)
    rem = ns - 3
    mids = []
    while rem > 0:
        g = min(7, rem)
        mids.append(g)
        rem -= g
    mids.sort(reverse=True)
    return [1] + mids + [2]


def _chunk_ns(rs):
    """Sweeps per output chunk: matmul PSUM writes may only start at
    partition 0/32/64, so successive sweeps' row offsets must stay in
    that set (and total rows within 128)."""
    k = 1
    while (k * rs) in (32, 64) and (k + 1) * rs <= 128:
        k += 1
    return k


def _basis():
    lc = np.arange(_TW, dtype=np.float32) - (_TW - 1) / 2.0
    lr = np.arange(_TH, dtype=np.float32) - (_TH - 1) / 2.0
    xl = np.tile(lc, _TH)                  # pixel p = row*_TW + col
    yl = np.repeat(lr, _TW)
    B = np.stack(
        [xl * xl, xl * yl, yl * yl, xl, yl, np.ones(_AREA, np.float32)], 0
    )
    return np.concatenate([B, B], axis=0)  # [12, 64]


def _build_core_data(core, m, ia, ib, ic, cl, tiles, ns, slotcap):
    """Host tensors for one core (layout shaped only by ns/slotcap)."""
    bf = _bf16dt()
    rs = ((3 * slotcap + 31) // 32) * 32   # PSUM partition offsets: 32-aligned
    sweeps = _pack_tiles(tiles, ns, slotcap)

    A = np.zeros((12, _AREA + ns * _CAP), np.float32)
    A[5, :] = _PAD_Q
    A[:, :_AREA] = _basis()
    mask = np.zeros((128, ns * _CAP), np.float32)
    colors = np.zeros((128, ns * rs), np.float32)
    rowmap = [None] * _NTILES

    y_base = core * _BAND
    for s, sw in enumerate(sweeps):
        for j, (t, idx, start) in enumerate(sw):
            rowmap[t] = s * rs + 3 * j
            n = len(idx)
            if n == 0:
                continue
            ty, tx = divmod(t, _TILES_X)
            cx = tx * _TW + (_TW - 1) / 2.0 + 0.5
            cy = y_base + ty * _TH + (_TH - 1) / 2.0 + 0.5
            mxl = m[idx, 0] - cx
            myl = m[idx, 1] - cy
            g_ia, g_ib, g_ic = ia[idx], ib[idx], ic[idx]
            coef = np.stack(
                [
                    g_ia,
                    2.0 * g_ib,
                    g_ic,
                    -2.0 * (g_ia * mxl + g_ib * myl),
                    -2.0 * (g_ib * mxl + g_ic * myl),
                    g_ia * mxl * mxl + 2.0 * g_ib * mxl * myl
                    + g_ic * myl * myl,
                ],
                axis=0,
            )  # [6, n] float64
            hi = coef.astype(bf)
            lo = (coef - hi.astype(np.float64)).astype(bf)
            c0 = _AREA + s * _CAP + start
            A[:6, c0 : c0 + n] = hi.astype(np.float32)
            A[6:, c0 : c0 + n] = lo.astype(np.float32)
            mask[start : start + n, s * _CAP + start : s * _CAP + start + n] = (
                np.triu(np.ones((n, n), np.float32), 1)
            )
            cc = s * rs + 3 * j
            colors[start : start + n, cc : cc + 3] = cl[idx]
    data = {
        "AB": A.astype(bf),
        "maskl": mask.astype(bf),
        "colors": colors.astype(bf),
    }
    return data, rowmap


def _patch_act_tables():
    import concourse.bacc as bacc
    import concourse.hw_specs as hw_specs
    from concourse import mybir

    # Exp and Ln alternate; make the act-table chooser satisfy both from the
    # combined table (one load instead of a ~1.3us reload per switch).
    if not getattr(hw_specs, "_gs_act_patch", False):
        _orig_get_tables = hw_specs.get_activation_tables

        def _patched(arch):
            tables = _orig_get_tables(arch)
            for name, funcs in tables.items():
                if name != "natural_log_exp_and_others":
                    funcs.discard(mybir.ActivationFunctionType.Exp)
                    funcs.discard(mybir.ActivationFunctionType.Ln)
            return tables

        hw_specs.get_activation_tables = _patched
        bacc.get_activation_tables = _patched
        hw_specs._gs_act_patch = True


def _build_program(ns, slotcap):
    from contextlib import ExitStack

    import concourse.bacc as bacc
    import concourse.tile as tile
    from concourse import mybir

    _patch_act_tables()

    F32 = mybir.dt.float32
    BF16 = mybir.dt.bfloat16
    AF = mybir.ActivationFunctionType
    OP = mybir.AluOpType

    rs = ((3 * slotcap + 31) // 32) * 32  # PSUM partition offsets: 32-aligned
    chunk_ns = _chunk_ns(rs)          # sweeps per output chunk
    groups = _split_groups(ns)
    gbase = np.cumsum([0] + groups)       # first sweep of each group
    G = len(groups)
    chunks = [
        list(range(c0, min(c0 + chunk_ns, ns)))
        for c0 in range(0, ns, chunk_ns)
    ]
    # pairs of chunks share one PSUM tile side by side (halves the number
    # of evacuation copies); a pair is evacuated when its last chunk's
    # group finishes
    pairs = [list(range(p0, min(p0 + 2, len(chunks))))
             for p0 in range(0, len(chunks), 2)]

    def grp_of(s):
        return int(np.searchsorted(gbase, s, side="right") - 1)

    pairs_by_group = {}
    for pi, pr in enumerate(pairs):
        last_sweep = chunks[pr[-1]][-1]
        pairs_by_group.setdefault(grp_of(last_sweep), []).append(pi)

    nc = bacc.Bacc(trn_type="TRN2", target_bir_lowering=False, debug=False)
    t_AB = nc.dram_tensor("AB", [12, _AREA + ns * _CAP], BF16,
                          kind="ExternalInput")
    t_mask = nc.dram_tensor("maskl", [128, ns * _CAP], BF16,
                            kind="ExternalInput")
    t_col = nc.dram_tensor("colors", [128, ns * rs], BF16,
                           kind="ExternalInput")
    nchunk = -(-ns // chunk_ns)
    t_out = nc.dram_tensor("out", [128, nchunk * _AREA], F32,
                           kind="ExternalOutput")

    with ExitStack() as ctx:
        tc = ctx.enter_context(tile.TileContext(nc))
        const = ctx.enter_context(tc.tile_pool(name="const", bufs=1))
        sb = ctx.enter_context(tc.tile_pool(name="sb", bufs=3))
        osb = ctx.enter_context(tc.tile_pool(name="osb", bufs=8))
        psq = ctx.enter_context(tc.tile_pool(name="psq", bufs=2, space="PSUM"))
        pst = ctx.enter_context(tc.tile_pool(name="pst", bufs=2, space="PSUM"))
        psi = ctx.enter_context(tc.tile_pool(name="psi", bufs=3, space="PSUM"))
        psw = ctx.enter_context(tc.tile_pool(name="psw", bufs=1, space="PSUM"))

        AB_all = const.tile([12, _AREA + ns * _CAP], BF16)
        nchunks_t = len(chunks)
        o_all = const.tile([128, nchunks_t * _AREA], F32)
        nc.vector.memset(o_all[:, (nchunks_t - 1) * _AREA :], 0.0)
        mask_all = const.tile([128, ns * _CAP], BF16)
        col_all = const.tile([128, ns * rs], BF16)

        # Input DMAs. Pool queue delivers with the lowest latency: it gets
        # basis+A of group 0 and the first two mask groups; SP streams the
        # remaining A per group, then colors, then late masks.
        gcut = min(2, G)
        acut = _AREA + int(gbase[gcut]) * _CAP
        nc.gpsimd.dma_start(AB_all[:, :acut], t_AB[:, :acut])
        for g in range(gcut, G):
            a0 = _AREA + int(gbase[g]) * _CAP
            a1 = _AREA + int(gbase[g + 1]) * _CAP
            nc.sync.dma_start(AB_all[:, a0:a1], t_AB[:, a0:a1])
        mcut = int(gbase[min(2, G)]) * _CAP   # masks for first two groups
        nc.gpsimd.dma_start(mask_all[:, :mcut], t_mask[:, :mcut])
        nc.sync.dma_start(col_all[:], t_col[:])
        if mcut < ns * _CAP:
            nc.sync.dma_start(mask_all[:, mcut:], t_mask[:, mcut:])

        basis = AB_all[:, :_AREA]

        # warm the PE clock while input DMAs are in flight
        warm = const.tile([128, 16], F32)
        nc.vector.memset(warm[:], 0.0)
        warm_ps = psw.tile([128, 16], F32)
        for _ in range(14):
            nc.tensor.matmul(
                warm_ps[:16, :16], warm[:], warm[:, :16], start=True, stop=True
            )

        # per-group tiles, created lazily at emission
        q_t = [None] * G
        e_t = [None] * G
        m_t = [None] * G
        al_t = [None] * G
        l_t = [None] * G
        tl_t = [None] * G
        T_t = [None] * G
        w_t = [None] * G
        img_t = [None] * len(pairs)
        Cg = [groups[g] * _AREA for g in range(G)]

        def stage(g, st):
            g0, g1 = int(gbase[g]), int(gbase[g + 1])
            C = Cg[g]
            if st == 0:
                q_t[g] = psq.tile([128, 512], F32, tag="q", name="q")
                for i, s in enumerate(range(g0, g1)):
                    nc.tensor.matmul(
                        q_t[g][:, i * _AREA : (i + 1) * _AREA],
                        AB_all[:, _AREA + s * _CAP : _AREA + (s + 1) * _CAP],
                        basis,
                        start=True,
                        stop=True,
                    )
            elif st == 1:
                e_t[g] = sb.tile([128, 512], F32, tag="e", name="e")
                nc.scalar.activation(
                    e_t[g][:, :C], q_t[g][:, :C], AF.Exp, scale=-0.5
                )
            elif st == 2:
                m_t[g] = sb.tile([128, 512], F32, tag="m", name="m")
                nc.vector.tensor_scalar(
                    m_t[g][:, :C], q_t[g][:, :C], _QTH, None, OP.is_le
                )
                al_t[g] = sb.tile([128, 512], F32, tag="al", name="al")
                nc.vector.scalar_tensor_tensor(
                    al_t[g][:, :C], e_t[g][:, :C], 0.99, m_t[g][:, :C],
                    OP.min, OP.mult,
                )
            elif st == 3:
                l_t[g] = sb.tile([128, 512], BF16, tag="l", name="l")
                nc.scalar.activation(
                    l_t[g][:, :C], al_t[g][:, :C], AF.Ln, bias=1.0, scale=-1.0
                )
            elif st == 4:
                tl_t[g] = pst.tile([128, 512], F32, tag="tl", name="tl")
                for i, s in enumerate(range(g0, g1)):
                    nc.tensor.matmul(
                        tl_t[g][:, i * _AREA : (i + 1) * _AREA],
                        mask_all[:, s * _CAP : (s + 1) * _CAP],
                        l_t[g][:, i * _AREA : (i + 1) * _AREA],
                        start=True,
                        stop=True,
                    )
            elif st == 5:
                T_t[g] = sb.tile([128, 512], F32, tag="T", name="T")
                nc.scalar.activation(T_t[g][:, :C], tl_t[g][:, :C], AF.Exp)
            elif st == 6:
                w_t[g] = sb.tile([128, 512], BF16, tag="w", name="w")
                nc.gpsimd.tensor_tensor(
                    w_t[g][:, :C], al_t[g][:, :C], T_t[g][:, :C], OP.mult
                )
            elif st == 7:
                for i, s in enumerate(range(g0, g1)):
                    ci = s // chunk_ns
                    pi = ci // 2
                    si = s - ci * chunk_ns
                    if img_t[pi] is None:
                        img_t[pi] = psi.tile(
                            [128, 2 * _AREA], F32, tag="img", name="img"
                        )
                        # zero regions img matmuls won't write (short last
                        # chunk / odd pair) so the pair evac reads defined data
                        for cj in pairs[pi]:
                            used = len(chunks[cj]) * rs
                            cc = (cj % 2) * _AREA
                            if used < 128:
                                nc.vector.memset(
                                    img_t[pi][used:, cc : cc + _AREA], 0.0
                                )
                        if len(pairs[pi]) == 1:
                            nc.vector.memset(img_t[pi][:, _AREA:], 0.0)
                    co = (ci % 2) * _AREA
                    nc.tensor.matmul(
                        img_t[pi][si * rs : (si + 1) * rs, co : co + _AREA],
                        col_all[:, s * rs : (s + 1) * rs],
                        w_t[g][:, i * _AREA : (i + 1) * _AREA],
                        start=True,
                        stop=True,
                    )
                for pi in pairs_by_group.get(g, []):
                    pw = len(pairs[pi]) * _AREA
                    p0 = pairs[pi][0] * _AREA
                    dst = o_all[:, p0 : p0 + pw]
                    last = pi == len(pairs) - 1
                    # GPSIMD cannot read PSUM on HW; evacuate on DVE, except
                    # the second-to-last pair may ride ACT (its queue slot
                    # is after the final Exp) to overlap the tail's copies.
                    if g >= G - 2 and not last and pi == len(pairs) - 2:
                        nc.scalar.copy(dst, img_t[pi][:, :pw])
                    else:
                        nc.vector.tensor_copy(dst, img_t[pi][:, :pw])
                    if pi == len(pairs) - 3:
                        nc.sync.dma_start(
                            t_out[:, : p0 + pw],
                            o_all[:, : p0 + pw],
                        )
                    elif last:
                        # both output DMAs ride SP: the exit barrier drains
                        # ~300ns faster when the last DMA completes on SP
                        # (Pool coordinates the barrier and pays two extra
                        # gather rounds if its own drain finishes last)
                        c0 = pairs[max(0, len(pairs) - 2)][0] * _AREA
                        nc.sync.dma_start(
                            t_out[:, c0:], o_all[:, c0:]
                        )

        for t in range(G + 8):
            for g in range(min(t, G - 1), max(-1, t - 8), -1):
                stage(g, t - g)

    nc.compile()
    return nc


def kernel(means_2d, covs_2d, depth_features, color_features, height, width):
    H, W = int(height), int(width)
    means_2d = np.asarray(means_2d, np.float32)
    covs_2d = np.asarray(covs_2d, np.float32)
    depth_features = np.asarray(depth_features, np.float32)
    color_features = np.asarray(color_features, np.float32)

    a, b, c = (
        covs_2d[:, 0].astype(np.float64),
        covs_2d[:, 1].astype(np.float64),
        covs_2d[:, 2].astype(np.float64),
    )
    det = a * c - b * b
    if H != _H or W != _W or np.any(det <= 0) or np.any(a <= 0) or np.any(c <= 0):
        return _reference_numpy(
            means_2d, covs_2d, depth_features, color_features, H, W
        )

    order = np.argsort(depth_features, kind="stable")
    m = means_2d[order].astype(np.float64)
    cvo = covs_2d[order].astype(np.float64)
    cl = color_features[order].astype(np.float32)
    a, b, c = cvo[:, 0], cvo[:, 1], cvo[:, 2]
    det = a * c - b * b
    ia, ib, ic = c / det, -b / det, a / det
    rx = np.sqrt(_QTH * a) + 1e-3
    ry = np.sqrt(_QTH * c) + 1e-3

    try:
        per_core_tiles = [
            _prep_core(core, m, ia, ib, ic, rx, ry) for core in range(_NCORES)
        ]
        ns, slotcap = _choose_shape(per_core_tiles)
        in_maps = []
        rowmaps = []
        for core in range(_NCORES):
            data, rowmap = _build_core_data(
                core, m, ia, ib, ic, cl, per_core_tiles[core], ns, slotcap
            )
            in_maps.append(data)
            rowmaps.append(rowmap)
    except ValueError:
        return _reference_numpy(
            means_2d, covs_2d, depth_features, color_features, H, W
        )

    nc = _build_program(ns, slotcap)
    if os.environ.get("GS_KERNEL_SIM") == "1":
        from types import SimpleNamespace

        from concourse.bass_interp import CoreSim

        results = []
        for core in range(_NCORES):
            sim = CoreSim(nc)
            for k, v in in_maps[core].items():
                sim.tensor(k)[:] = v
            sim.simulate()
            results.append({"out": np.array(sim.tensor("out"))})
        res = SimpleNamespace(results=results)
    else:
        from concourse.bass_utils import run_bass_kernel_spmd

        res = run_bass_kernel_spmd(nc, in_maps, core_ids=list(range(_NCORES)))

    rs = ((3 * slotcap + 31) // 32) * 32
    chunk_ns = _chunk_ns(rs)
    img = np.zeros((3, _H, _W), np.float32)
    for core in range(_NCORES):
        o = res.results[core]["out"]          # [128, nchunk*64]
        rowmap = rowmaps[core]
        for t in range(_NTILES):
            ty, tx = divmod(t, _TILES_X)
            vr = rowmap[t]                    # virtual row s*rs + 3j
            s = vr // rs
            c = s // chunk_ns
            r = vr - c * chunk_ns * rs
            blk = np.asarray(
                o[r : r + 3, c * _AREA : (c + 1) * _AREA], np.float32
            ).reshape(3, _TH, _TW)
            img[
                :,
                core * _BAND + ty * _TH : core * _BAND + (ty + 1) * _TH,
                tx * _TW : (tx + 1) * _TW,
            ] = blk
    return img


# revision 6
# speedup vs baseline: 1.0093x; 1.0093x over previous
"""Differentiable Gaussian-splat tile compositor on 8 Trainium2 cores, v2.

Sharding: image split into 8 horizontal bands (32 rows), one per NeuronCore.
Within a band, 8x8 pixel tiles (128 tiles); each Gaussian is assigned
(host-side, exact per-pixel-center test) to the tiles where it can reach
alpha >= 1/255 (q <= 2*ln 255). Tiles are bin-packed into NS sweeps of <=128
depth-ordered gaussian rows and <= SLOTCAP tiles; all per-sweep layout is
padded to SLOTCAP so the device program is identical across cores (SPMD) --
the block-diagonal strict-lower mask realizing each tile's exclusive
cumulative sum of ln(1-alpha) is DMA'd as data.

Sweeps are batched into groups (<=512 PSUM cols). Per group g:
  q[g,pix] = A_s[12,128]^T @ B[12,64]   per sweep (PE, bf16 hi/lo split; the
                                         8x8 tile-local basis is exact bf16)
  m        = q <= 2 ln 255              (DVE, runs parallel with Exp)
  e        = exp(-q/2)                  (ACT)
  alpha    = min(e,.99) * m             (DVE fused)
  l        = ln(1 - alpha)              (ACT free affine, bf16 out)
  Tlog     = StrictLowerBlockDiag @ l   (PE per sweep, bf16)
  T        = exp(Tlog)                  (ACT)
  w        = alpha * T                  (Pool, bf16 out)
  img      = Colors_s^T @ w             (PE per sweep, partition-offset rows)
Per img chunk (<=128 output rows): evacuation copy + output DMA, overlapped
with later groups. Emission is software-pipelined (stage skew) so each
engine queue has lookahead; groups are sized small-big-small so the fill and
tail of the ACT-bound stream stay short.
"""

import os
import numpy as np

_H = 256
_W = 256
_NCORES = 8
_TH = 8
_TW = 8
_AREA = _TH * _TW
_BAND = _H // _NCORES
_TILES_X = _W // _TW
_TILES_Y = _BAND // _TH
_NTILES = _TILES_X * _TILES_Y
_CAP = 128
_QTH = float(2.0 * np.log(255.0))
_PAD_Q = 128.0


def _bf16dt():
    import ml_dtypes

    return ml_dtypes.bfloat16


def _reference_numpy(means_2d, covs_2d, depth_features, color_features, H, W):
    """Exact slow fallback (mirrors reference.py math)."""
    order = np.argsort(depth_features, kind="stable")
    m = means_2d[order].astype(np.float32)
    cv = covs_2d[order].astype(np.float32)
    cl = color_features[order].astype(np.float32)
    a, b, c = cv[:, 0], cv[:, 1], cv[:, 2]
    det = a * c - b * b
    ia, ib, ic = c / det, -b / det, a / det
    xs = np.arange(W, dtype=np.float32) + 0.5
    ys = np.arange(H, dtype=np.float32) + 0.5
    img = np.zeros((3, H, W), np.float32)
    T = np.ones((H, W), np.float32)
    for p in range(m.shape[0]):
        dx = xs[None, :] - m[p, 0]
        dy = ys[:, None] - m[p, 1]
        q = ia[p] * dx * dx + 2.0 * ib[p] * dx * dy + ic[p] * dy * dy
        alpha = np.minimum(np.float32(0.99), np.exp(np.float32(-0.5) * q))
        alpha = np.where(alpha < 1.0 / 255.0, np.float32(0.0), alpha)
        w = alpha * T
        img += cl[p][:, None, None] * w[None]
        T = T * (1.0 - alpha)
    return img


def _prep_core(core, m, ia, ib, ic, rx, ry):
    """Per-tile depth-ordered gaussian index lists for one core band."""
    tiles = []
    y_base = core * _BAND
    for ty in range(_TILES_Y):
        y0 = y_base + ty * _TH
        for tx in range(_TILES_X):
            x0 = tx * _TW
            cand = np.nonzero(
                (m[:, 0] + rx >= x0 + 0.5 - 1e-6)
                & (m[:, 0] - rx <= x0 + _TW - 0.5 + 1e-6)
                & (m[:, 1] + ry >= y0 + 0.5 - 1e-6)
                & (m[:, 1] - ry <= y0 + _TH - 0.5 + 1e-6)
            )[0]
            if cand.size:
                dx = (x0 + 0.5 + np.arange(_TW))[None, :] - m[cand, 0][:, None]
                dy = (y0 + 0.5 + np.arange(_TH))[None, :] - m[cand, 1][:, None]
                q = (
                    ia[cand][:, None, None] * (dx * dx)[:, None, :]
                    + 2.0 * ib[cand][:, None, None]
                    * dx[:, None, :] * dy[:, :, None]
                    + ic[cand][:, None, None] * (dy * dy)[:, :, None]
                )
                qmin = q.reshape(cand.size, -1).min(axis=1)
                cand = cand[qmin <= _QTH + 1e-3]
            tiles.append(cand)
    return tiles


def _pack_tiles(tiles, n_sweeps, slotcap):
    """FFD into exactly n_sweeps bins of <=_CAP rows, <=slotcap tiles.
    Returns sweeps: list of lists of (tile, idx, start_row)."""
    order = sorted(range(len(tiles)), key=lambda i: -len(tiles[i]))
    sweeps = [[] for _ in range(n_sweeps)]
    loads = [0] * n_sweeps
    for i in order:
        n = len(tiles[i])
        # best-fit: fullest bin that still fits (rows); balances slots via cap
        best, best_load = None, -1
        for s in range(n_sweeps):
            if loads[s] + n <= _CAP and len(sweeps[s]) < slotcap:
                if loads[s] > best_load:
                    best, best_load = s, loads[s]
        if best is None:
            raise ValueError("packing overflow")
        sweeps[best].append((i, tiles[i], loads[best]))
        loads[best] += n
    return sweeps


def _choose_shape(per_core_tiles):
    """Smallest (n_sweeps, slotcap) feasible for every core."""
    lo = max(
        max(1, -(-sum(len(t) for t in tiles) // _CAP))
        for tiles in per_core_tiles
    )
    for ns in range(lo, 65):
        for cap in range(-(-_NTILES // ns), 43):
            try:
                for tiles in per_core_tiles:
                    _pack_tiles(tiles, ns, cap)
                return ns, cap
            except ValueError:
                continue
    raise ValueError("unpackable")


def _split_groups(ns):
    """Consecutive groups of <=8 sweeps (<=512 PSUM cols). First and last
    are a single sweep: the first keeps the initial Exp's wait inline (so
    the act-table load stays at the head of the ACT queue) and starts the
    stream as soon as the table lands; the last keeps the tail chain
    short."""
    ov = os.environ.get("GS_GROUPS")
    if ov:
        groups = [int(x) for x in ov.split(",")]
        if sum(groups) == ns and all(1 <= g <= 8 for g in groups):
            return groups
    if ns <= 2:
        return [ns]
    if ns <= 5:
        return [1] + [ns - 2] + [1]
    # head group of 1 (fill), tail group of 2 (short drain chain),
    # middles of <=7 sweeps (balance per-op overhead vs pipeline slack)
    rem = ns - 3
    mids = []
    while rem > 0:
        g = min(7, rem)
        mids.append(g)
        rem -= g
    mids.sort(reverse=True)
    return [1] + mids + [2]


def _chunk_ns(rs):
    """Sweeps per output chunk: matmul PSUM writes may only start at
    partition 0/32/64, so successive sweeps' row offsets must stay in
    that set (and total rows within 128)."""
    k = 1
    while (k * rs) in (32, 64) and (k + 1) * rs <= 128:
        k += 1
    return k


def _basis():
    lc = np.arange(_TW, dtype=np.float32) - (_TW - 1) / 2.0
    lr = np.arange(_TH, dtype=np.float32) - (_TH - 1) / 2.0
    xl = np.tile(lc, _TH)                  # pixel p = row*_TW + col
    yl = np.repeat(lr, _TW)
    B = np.stack(
        [xl * xl, xl * yl, yl * yl, xl, yl, np.ones(_AREA, np.float32)], 0
    )
    return np.concatenate([B, B], axis=0)  # [12, 64]


def _build_core_data(core, m, ia, ib, ic, cl, tiles, ns, slotcap):
    """Host tensors for one core (layout shaped only by ns/slotcap)."""
    bf = _bf16dt()
    rs = ((3 * slotcap + 31) // 32) * 32   # PSUM partition offsets: 32-aligned
    sweeps = _pack_tiles(tiles, ns, slotcap)

    A = np.zeros((12, _AREA + ns * _CAP), np.float32)
    A[5, :] = _PAD_Q
    A[:, :_AREA] = _basis()
    mask = np.zeros((128, ns * _CAP), np.float32)
    colors = np.zeros((128, ns * rs), np.float32)
    rowmap = [None] * _NTILES

    y_base = core * _BAND
    for s, sw in enumerate(sweeps):
        for j, (t, idx, start) in enumerate(sw):
            rowmap[t] = s * rs + 3 * j
            n = len(idx)
            if n == 0:
                continue
            ty, tx = divmod(t, _TILES_X)
            cx = tx * _TW + (_TW - 1) / 2.0 + 0.5
            cy = y_base + ty * _TH + (_TH - 1) / 2.0 + 0.5
            mxl = m[idx, 0] - cx
            myl = m[idx, 1] - cy
            g_ia, g_ib, g_ic = ia[idx], ib[idx], ic[idx]
            coef = np.stack(
                [
                    g_ia,
                    2.0 * g_ib,
                    g_ic,
                    -2.0 * (g_ia * mxl + g_ib * myl),
                    -2.0 * (g_ib * mxl + g_ic * myl),
                    g_ia * mxl * mxl + 2.0 * g_ib * mxl * myl
                    + g_ic * myl * myl,
                ],
                axis=0,
            )  # [6, n] float64
            hi = coef.astype(bf)
            lo = (coef - hi.astype(np.float64)).astype(bf)
            c0 = _AREA + s * _CAP + start
            A[:6, c0 : c0 + n] = hi.astype(np.float32)
            A[6:, c0 : c0 + n] = lo.astype(np.float32)
            mask[start : start + n, s * _CAP + start : s * _CAP + start + n] = (
                np.triu(np.ones((n, n), np.float32), 1)
            )
            cc = s * rs + 3 * j
            colors[start : start + n, cc : cc + 3] = cl[idx]
    data = {
        "AB": A.astype(bf),
        "maskl": mask.astype(bf),
        "colors": colors.astype(bf),
    }
    return data, rowmap


def _patch_act_tables():
    import concourse.bacc as bacc
    import concourse.hw_specs as hw_specs
    from concourse import mybir

    # Exp and Ln alternate; make the act-table chooser satisfy both from the
    # combined table (one load instead of a ~1.3us reload per switch).
    if not getattr(hw_specs, "_gs_act_patch", False):
        _orig_get_tables = hw_specs.get_activation_tables

        def _patched(arch):
            tables = _orig_get_tables(arch)
            for name, funcs in tables.items():
                if name != "natural_log_exp_and_others":
                    funcs.discard(mybir.ActivationFunctionType.Exp)
                    funcs.discard(mybir.ActivationFunctionType.Ln)
            return tables

        hw_specs.get_activation_tables = _patched
        bacc.get_activation_tables = _patched
        hw_specs._gs_act_patch = True


def _build_program(ns, slotcap):
    from contextlib import ExitStack

    import concourse.bacc as bacc
    import concourse.tile as tile
    from concourse import mybir

    _patch_act_tables()

    F32 = mybir.dt.float32
    BF16 = mybir.dt.bfloat16
    AF = mybir.ActivationFunctionType
    OP = mybir.AluOpType

    rs = ((3 * slotcap + 31) // 32) * 32  # PSUM partition offsets: 32-aligned
    chunk_ns = _chunk_ns(rs)          # sweeps per output chunk
    groups = _split_groups(ns)
    gbase = np.cumsum([0] + groups)       # first sweep of each group
    G = len(groups)
    chunks = [
        list(range(c0, min(c0 + chunk_ns, ns)))
        for c0 in range(0, ns, chunk_ns)
    ]
    # pairs of chunks share one PSUM tile side by side (halves the number
    # of evacuation copies); a pair is evacuated when its last chunk's
    # group finishes
    pairs = [list(range(p0, min(p0 + 2, len(chunks))))
             for p0 in range(0, len(chunks), 2)]

    def grp_of(s):
        return int(np.searchsorted(gbase, s, side="right") - 1)

    pairs_by_group = {}
    for pi, pr in enumerate(pairs):
        last_sweep = chunks[pr[-1]][-1]
        pairs_by_group.setdefault(grp_of(last_sweep), []).append(pi)

    nc = bacc.Bacc(trn_type="TRN2", target_bir_lowering=False, debug=False)
    t_AB = nc.dram_tensor("AB", [12, _AREA + ns * _CAP], BF16,
                          kind="ExternalInput")
    t_mask = nc.dram_tensor("maskl", [128, ns * _CAP], BF16,
                            kind="ExternalInput")
    t_col = nc.dram_tensor("colors", [128, ns * rs], BF16,
                           kind="ExternalInput")
    nchunk = -(-ns // chunk_ns)
    t_out = nc.dram_tensor("out", [128, nchunk * _AREA], F32,
                           kind="ExternalOutput")

    with ExitStack() as ctx:
        tc = ctx.enter_context(tile.TileContext(nc))
        const = ctx.enter_context(tc.tile_pool(name="const", bufs=1))
        sb = ctx.enter_context(tc.tile_pool(name="sb", bufs=3))
        osb = ctx.enter_context(tc.tile_pool(name="osb", bufs=8))
        psq = ctx.enter_context(tc.tile_pool(name="psq", bufs=2, space="PSUM"))
        pst = ctx.enter_context(tc.tile_pool(name="pst", bufs=2, space="PSUM"))
        psi = ctx.enter_context(tc.tile_pool(name="psi", bufs=2, space="PSUM"))
        psq2 = ctx.enter_context(tc.tile_pool(name="psq2", bufs=1, space="PSUM"))
        psw = ctx.enter_context(tc.tile_pool(name="psw", bufs=1, space="PSUM"))

        AB_all = const.tile([12, _AREA + ns * _CAP], BF16)
        nchunks_t = len(chunks)
        o_all = const.tile([128, nchunks_t * _AREA], F32)
        nc.vector.memset(o_all[:, (nchunks_t - 1) * _AREA :], 0.0)
        mask_all = const.tile([128, ns * _CAP], BF16)
        col_all = const.tile([128, ns * rs], BF16)

        # Input DMAs. Pool queue delivers with the lowest latency: it gets
        # basis+A of group 0 and the first two mask groups; SP streams the
        # remaining A per group, then colors, then late masks.
        gcut = min(2, G)
        acut = _AREA + int(gbase[gcut]) * _CAP
        nc.gpsimd.dma_start(AB_all[:, :acut], t_AB[:, :acut])
        for g in range(gcut, G):
            a0 = _AREA + int(gbase[g]) * _CAP
            a1 = _AREA + int(gbase[g + 1]) * _CAP
            nc.sync.dma_start(AB_all[:, a0:a1], t_AB[:, a0:a1])
        mcut = int(gbase[min(2, G)]) * _CAP   # masks for first two groups
        nc.gpsimd.dma_start(mask_all[:, :mcut], t_mask[:, :mcut])
        nc.sync.dma_start(col_all[:], t_col[:])
        if mcut < ns * _CAP:
            nc.sync.dma_start(mask_all[:, mcut:], t_mask[:, mcut:])

        basis = AB_all[:, :_AREA]

        # warm the PE clock while input DMAs are in flight
        warm = const.tile([128, 16], F32)
        nc.vector.memset(warm[:], 0.0)
        warm_ps = psw.tile([128, 16], F32)
        for _ in range(14):
            nc.tensor.matmul(
                warm_ps[:16, :16], warm[:], warm[:, :16], start=True, stop=True
            )

        # per-group tiles, created lazily at emission
        q_t = [None] * G
        q2_t = [None] * G
        e_t = [None] * G
        m_t = [None] * G
        al_t = [None] * G
        l_t = [None] * G
        tl_t = [None] * G
        T_t = [None] * G
        w_t = [None] * G
        img_t = [None] * len(pairs)
        Cg = [groups[g] * _AREA for g in range(G)]

        def stage(g, st):
            g0, g1 = int(gbase[g]), int(gbase[g + 1])
            C = Cg[g]
            if st == 0:
                q_t[g] = psq.tile([128, 512], F32, tag="q", name="q")
                for i, s in enumerate(range(g0, g1)):
                    nc.tensor.matmul(
                        q_t[g][:, i * _AREA : (i + 1) * _AREA],
                        AB_all[:, _AREA + s * _CAP : _AREA + (s + 1) * _CAP],
                        basis,
                        start=True,
                        stop=True,
                    )
                if g == 1 and G >= 3:
                    # group 1's threshold mask paces the whole DVE chain;
                    # give it a private q copy so it isn't reader-serialized
                    # behind Exp (same-tile readers serialize in order)
                    q2_t[g] = psq2.tile([128, 512], F32, tag="q2", name="q2")
                    for i, s in enumerate(range(g0, g1)):
                        nc.tensor.matmul(
                            q2_t[g][:, i * _AREA : (i + 1) * _AREA],
                            AB_all[:, _AREA + s * _CAP
                                   : _AREA + (s + 1) * _CAP],
                            basis,
                            start=True,
                            stop=True,
                        )
            elif st == 1:
                e_t[g] = sb.tile([128, 512], F32, tag="e", name="e")
                nc.scalar.activation(
                    e_t[g][:, :C], q_t[g][:, :C], AF.Exp, scale=-0.5
                )
            elif st == 2:
                m_t[g] = sb.tile([128, 512], F32, tag="m", name="m")
                msrc = q2_t[g] if q2_t[g] is not None else q_t[g]
                nc.vector.tensor_scalar(
                    m_t[g][:, :C], msrc[:, :C], _QTH, None, OP.is_le
                )
                al_t[g] = sb.tile([128, 512], F32, tag="al", name="al")
                nc.vector.scalar_tensor_tensor(
                    al_t[g][:, :C], e_t[g][:, :C], 0.99, m_t[g][:, :C],
                    OP.min, OP.mult,
                )
            elif st == 3:
                l_t[g] = sb.tile([128, 512], BF16, tag="l", name="l")
                nc.scalar.activation(
                    l_t[g][:, :C], al_t[g][:, :C], AF.Ln, bias=1.0, scale=-1.0
                )
            elif st == 4:
                tl_t[g] = pst.tile([128, 512], F32, tag="tl", name="tl")
                for i, s in enumerate(range(g0, g1)):
                    nc.tensor.matmul(
                        tl_t[g][:, i * _AREA : (i + 1) * _AREA],
                        mask_all[:, s * _CAP : (s + 1) * _CAP],
                        l_t[g][:, i * _AREA : (i + 1) * _AREA],
                        start=True,
                        stop=True,
                    )
            elif st == 5:
                T_t[g] = sb.tile([128, 512], F32, tag="T", name="T")
                nc.scalar.activation(T_t[g][:, :C], tl_t[g][:, :C], AF.Exp)
            elif st == 6:
                w_t[g] = sb.tile([128, 512], BF16, tag="w", name="w")
                nc.gpsimd.tensor_tensor(
                    w_t[g][:, :C], al_t[g][:, :C], T_t[g][:, :C], OP.mult
                )
            elif st == 7:
                for i, s in enumerate(range(g0, g1)):
                    ci = s // chunk_ns
                    pi = ci // 2
                    si = s - ci * chunk_ns
                    if img_t[pi] is None:
                        img_t[pi] = psi.tile(
                            [128, 2 * _AREA], F32, tag="img", name="img"
                        )
                        # zero regions img matmuls won't write (short last
                        # chunk / odd pair) so the pair evac reads defined data
                        for cj in pairs[pi]:
                            used = len(chunks[cj]) * rs
                            cc = (cj % 2) * _AREA
                            if used < 128:
                                nc.vector.memset(
                                    img_t[pi][used:, cc : cc + _AREA], 0.0
                                )
                        if len(pairs[pi]) == 1:
                            nc.vector.memset(img_t[pi][:, _AREA:], 0.0)
                    co = (ci % 2) * _AREA
                    nc.tensor.matmul(
                        img_t[pi][si * rs : (si + 1) * rs, co : co + _AREA],
                        col_all[:, s * rs : (s + 1) * rs],
                        w_t[g][:, i * _AREA : (i + 1) * _AREA],
                        start=True,
                        stop=True,
                    )
                for pi in pairs_by_group.get(g, []):
                    pw = len(pairs[pi]) * _AREA
                    p0 = pairs[pi][0] * _AREA
                    dst = o_all[:, p0 : p0 + pw]
                    last = pi == len(pairs) - 1
                    # GPSIMD cannot read PSUM on HW; evacuate on DVE, except
                    # the second-to-last pair may ride ACT (its queue slot
                    # is after the final Exp) to overlap the tail's copies.
                    if g >= G - 2 and not last and pi == len(pairs) - 2:
                        nc.scalar.copy(dst, img_t[pi][:, :pw])
                    else:
                        nc.vector.tensor_copy(dst, img_t[pi][:, :pw])
                    if pi == len(pairs) - 3:
                        nc.sync.dma_start(
                            t_out[:, : p0 + pw],
                            o_all[:, : p0 + pw],
                        )
                    elif last:
                        # both output DMAs ride SP: the exit barrier drains
                        # ~300ns faster when the last DMA completes on SP
                        # (Pool coordinates the barrier and pays two extra
                        # gather rounds if its own drain finishes last)
                        c0 = pairs[max(0, len(pairs) - 2)][0] * _AREA
                        nc.sync.dma_start(
                            t_out[:, c0:], o_all[:, c0:]
                        )

        for t in range(G + 8):
            for g in range(min(t, G - 1), max(-1, t - 8), -1):
                stage(g, t - g)

    nc.compile()
    return nc


def kernel(means_2d, covs_2d, depth_features, color_features, height, width):
    H, W = int(height), int(width)
    means_2d = np.asarray(means_2d, np.float32)
    covs_2d = np.asarray(covs_2d, np.float32)
    depth_features = np.asarray(depth_features, np.float32)
    color_features = np.asarray(color_features, np.float32)

    a, b, c = (
        covs_2d[:, 0].astype(np.float64),
        covs_2d[:, 1].astype(np.float64),
        covs_2d[:, 2].astype(np.float64),
    )
    det = a * c - b * b
    if H != _H or W != _W or np.any(det <= 0) or np.any(a <= 0) or np.any(c <= 0):
        return _reference_numpy(
            means_2d, covs_2d, depth_features, color_features, H, W
        )

    order = np.argsort(depth_features, kind="stable")
    m = means_2d[order].astype(np.float64)
    cvo = covs_2d[order].astype(np.float64)
    cl = color_features[order].astype(np.float32)
    a, b, c = cvo[:, 0], cvo[:, 1], cvo[:, 2]
    det = a * c - b * b
    ia, ib, ic = c / det, -b / det, a / det
    rx = np.sqrt(_QTH * a) + 1e-3
    ry = np.sqrt(_QTH * c) + 1e-3

    try:
        per_core_tiles = [
            _prep_core(core, m, ia, ib, ic, rx, ry) for core in range(_NCORES)
        ]
        ns, slotcap = _choose_shape(per_core_tiles)
        in_maps = []
        rowmaps = []
        for core in range(_NCORES):
            data, rowmap = _build_core_data(
                core, m, ia, ib, ic, cl, per_core_tiles[core], ns, slotcap
            )
            in_maps.append(data)
            rowmaps.append(rowmap)
    except ValueError:
        return _reference_numpy(
            means_2d, covs_2d, depth_features, color_features, H, W
        )

    nc = _build_program(ns, slotcap)
    if os.environ.get("GS_KERNEL_SIM") == "1":
        from types import SimpleNamespace

        from concourse.bass_interp import CoreSim

        results = []
        for core in range(_NCORES):
            sim = CoreSim(nc)
            for k, v in in_maps[core].items():
                sim.tensor(k)[:] = v
            sim.simulate()
            results.append({"out": np.array(sim.tensor("out"))})
        res = SimpleNamespace(results=results)
    else:
        from concourse.bass_utils import run_bass_kernel_spmd

        res = run_bass_kernel_spmd(nc, in_maps, core_ids=list(range(_NCORES)))

    rs = ((3 * slotcap + 31) // 32) * 32
    chunk_ns = _chunk_ns(rs)
    img = np.zeros((3, _H, _W), np.float32)
    for core in range(_NCORES):
        o = res.results[core]["out"]          # [128, nchunk*64]
        rowmap = rowmaps[core]
        for t in range(_NTILES):
            ty, tx = divmod(t, _TILES_X)
            vr = rowmap[t]                    # virtual row s*rs + 3j
            s = vr // rs
            c = s // chunk_ns
            r = vr - c * chunk_ns * rs
            blk = np.asarray(
                o[r : r + 3, c * _AREA : (c + 1) * _AREA], np.float32
            ).reshape(3, _TH, _TW)
            img[
                :,
                core * _BAND + ty * _TH : core * _BAND + (ty + 1) * _TH,
                tx * _TW : (tx + 1) * _TW,
            ] = blk
    return img
